# revision 1
# baseline (speedup 1.0000x reference)
"""Trainium2 Bass kernel for nn_DistSAGE (3-layer GraphSAGE, mean aggregation).

Strategy: recursive data-parallel sharding over the 8 NeuronCores with zero
collectives. Each core owns 64 of the 512 output nodes; the host computes the
exact set of layer-1 / layer-0 rows that subtree needs (~360 / ~3650 rows) and
ships per-core index tables. Each core then:
  layer 0: indirect-DMA gathers self + 10 neighbor rows (f32, 4 KB each) from
           the full replicated x table per 128-dst chunk, tree-adds the
           neighbors on DVE, transposes [dst, feat] -> [feat, dst] chunks on
           the PE, and matmuls against [Wself ; Wneigh/10] into PSUM (+bias via
           a K=1 ones matmul), relu -> h0 scratch in DRAM.
  layers 1/2: same machinery over the small h0/h1 tables (D=256).
The output is dst-major so each core writes its 64x19 logits; the host
concatenates. Inputs are replicated (x, weights) or per-core (index tables).
"""

import numpy as np

_N0, _N1, _N2, _N3 = 256000, 25600, 2560, 512
_DIN, _DH, _DOUT = 1024, 256, 19
_F0, _F1, _F2 = 10, 10, 5
_NCORES = 8
_P = 128
_OUT_PER_CORE = _N3 // _NCORES  # 64

_compiled = {}


def _build(u0p, u1p):
    import concourse.bass as bass
    import concourse.mybir as mybir
    import concourse.tile as tile
    from concourse import bacc
    from concourse.masks import make_identity

    P = _P
    nc = bacc.Bacc(
        "TRN2", target_bir_lowering=False, debug=False, num_devices=_NCORES,
        num_swdge_queues=4,
    )
    f32 = mybir.dt.float32
    i32 = mybir.dt.int32

    x = nc.dram_tensor("x", [_N0, _DIN], f32, kind="ExternalInput")
    gidx0 = nc.dram_tensor("gidx0", [u0p, _F0 + 1], i32, kind="ExternalInput")
    gidx1 = nc.dram_tensor("gidx1", [u1p, _F1 + 1], i32, kind="ExternalInput")
    gidx2 = nc.dram_tensor("gidx2", [P, _F2 + 1], i32, kind="ExternalInput")
    wcat0 = nc.dram_tensor("wcat0", [2 * _DIN, _DH], f32, kind="ExternalInput")
    wcat1 = nc.dram_tensor("wcat1", [2 * _DH, _DH], f32, kind="ExternalInput")
    wcat2 = nc.dram_tensor("wcat2", [2 * _DH, _DOUT], f32, kind="ExternalInput")
    bias0 = nc.dram_tensor("bias0", [1, _DH], f32, kind="ExternalInput")
    bias1 = nc.dram_tensor("bias1", [1, _DH], f32, kind="ExternalInput")
    bias2 = nc.dram_tensor("bias2", [1, _DOUT], f32, kind="ExternalInput")
    out = nc.dram_tensor("out", [P, _DOUT], f32, kind="ExternalOutput")

    h0 = nc.dram_tensor("h0scratch", [u0p, _DH], f32, kind="Internal")
    h1 = nc.dram_tensor("h1scratch", [u1p, _DH], f32, kind="Internal")

    with tile.TileContext(nc) as tc:
        with (
            tc.tile_pool(name="const", bufs=1) as cpool,
            tc.tile_pool(name="gather", bufs=2) as gpool,
            tc.tile_pool(name="zt", bufs=2) as zpool,
            tc.tile_pool(name="outp", bufs=2) as opool,
            tc.tile_pool(name="psacc", bufs=2, space="PSUM") as psacc,
            tc.tile_pool(name="pstp", bufs=4, space="PSUM") as pstp,
        ):
            ident = cpool.tile([P, P], f32)
            make_identity(nc, ident[:])
            ones = cpool.tile([1, P], f32)
            nc.gpsimd.memset(ones[:], 1.0)

            # resident weights: k-chunk c of wcat lives at wt[:, c*DO:(c+1)*DO]
            def load_w(wdram, kd, do, name):
                wt = cpool.tile([P, kd // P * do], f32, name=name)
                for k in range(kd // P):
                    nc.sync.dma_start(
                        out=wt[:, k * do : (k + 1) * do],
                        in_=wdram[k * P : (k + 1) * P, :],
                    )
                return wt

            wt0 = load_w(wcat0, 2 * _DIN, _DH, "wt0")
            wt1 = load_w(wcat1, 2 * _DH, _DH, "wt1")
            wt2 = load_w(wcat2, 2 * _DH, _DOUT, "wt2")
            bt0 = cpool.tile([1, _DH], f32)
            nc.sync.dma_start(out=bt0[:], in_=bias0[:])
            bt1 = cpool.tile([1, _DH], f32)
            nc.sync.dma_start(out=bt1[:], in_=bias1[:])
            bt2 = cpool.tile([1, _DOUT], f32)
            nc.sync.dma_start(out=bt2[:], in_=bias2[:])

            def layer(src, src_dram_rows, idx_dram, nd, d, fan, wt, bt, do, relu,
                      dst, self_contig=False):
                g_width = (fan + 1) * d
                kc = 2 * d // P
                for c in range(nd // P):
                    idx_t = gpool.tile([P, fan + 1], i32, tag="idx")
                    nc.sync.dma_start(
                        out=idx_t[:], in_=idx_dram[c * P : (c + 1) * P, :]
                    )
                    g = gpool.tile([P, g_width], f32, tag=f"g{d}")
                    if self_contig:
                        # host orders the dst table so chunk c's self rows are
                        # rows [c*P, (c+1)*P) of src — plain DMA, no indirection
                        nc.sync.dma_start(
                            out=g[:, 0:d], in_=src[c * P : (c + 1) * P, :]
                        )
                    for j in range(0 if not self_contig else 1, fan + 1):
                        ins = nc.gpsimd.indirect_dma_start(
                            out=g[:, j * d : (j + 1) * d],
                            out_offset=None,
                            in_=src[:],
                            in_offset=bass.IndirectOffsetOnAxis(
                                ap=idx_t[:, j : j + 1], axis=0
                            ),
                        )
                        # spread gathers over the 4 SWDGE queues so descriptor
                        # generation/drain pipelines instead of serializing
                        if j % 4:
                            ins.ins.queue = f"qPoolDynamic{j % 4}"
                    # tree-sum the fan neighbor rows into g[:, d:2d]
                    if fan == 10:
                        nc.vector.tensor_add(
                            out=g[:, d : 6 * d], in0=g[:, d : 6 * d],
                            in1=g[:, 6 * d : 11 * d],
                        )
                        nc.vector.tensor_add(
                            out=g[:, d : 3 * d], in0=g[:, d : 3 * d],
                            in1=g[:, 3 * d : 5 * d],
                        )
                        nc.vector.tensor_add(
                            out=g[:, d : 2 * d], in0=g[:, d : 2 * d],
                            in1=g[:, 2 * d : 3 * d],
                        )
                        nc.vector.tensor_add(
                            out=g[:, d : 2 * d], in0=g[:, d : 2 * d],
                            in1=g[:, 5 * d : 6 * d],
                        )
                    elif fan == 5:
                        nc.vector.tensor_add(
                            out=g[:, d : 3 * d], in0=g[:, d : 3 * d],
                            in1=g[:, 3 * d : 5 * d],
                        )
                        nc.vector.tensor_add(
                            out=g[:, d : 2 * d], in0=g[:, d : 2 * d],
                            in1=g[:, 2 * d : 3 * d],
                        )
                        nc.vector.tensor_add(
                            out=g[:, d : 2 * d], in0=g[:, d : 2 * d],
                            in1=g[:, 5 * d : 6 * d],
                        )
                    else:
                        raise NotImplementedError(fan)

                    zt = zpool.tile([P, 2 * d], f32, tag=f"zt{d}")
                    for k in range(kc):
                        tp = pstp.tile([P, P], f32, tag="tp")
                        nc.tensor.transpose(
                            out=tp[:], in_=g[:, k * P : (k + 1) * P],
                            identity=ident[:],
                        )
                        nc.vector.tensor_copy(
                            out=zt[:, k * P : (k + 1) * P], in_=tp[:]
                        )

                    acc = psacc.tile([P, do], f32, tag="acc")
                    for k in range(kc):
                        nc.tensor.matmul(
                            out=acc[:],
                            lhsT=zt[:, k * P : (k + 1) * P],
                            rhs=wt[:, k * do : (k + 1) * do],
                            start=(k == 0),
                            stop=False,
                        )
                    nc.tensor.matmul(
                        out=acc[:], lhsT=ones[:], rhs=bt[:], start=False, stop=True
                    )
                    ot = opool.tile([P, do], f32, tag=f"ot{do}")
                    nc.scalar.activation(
                        out=ot[:],
                        in_=acc[:],
                        func=(
                            mybir.ActivationFunctionType.Relu
                            if relu
                            else mybir.ActivationFunctionType.Copy
                        ),
                    )
                    nc.sync.dma_start(out=dst[c * P : (c + 1) * P, :], in_=ot[:])

            layer(x, _N0, gidx0, u0p, _DIN, _F0, wt0, bt0, _DH, True, h0)
            layer(h0, u0p, gidx1, u1p, _DH, _F1, wt1, bt1, _DH, True, h1,
                  self_contig=True)
            layer(h1, u1p, gidx2, P, _DH, _F2, wt2, bt2, _DOUT, False, out,
                  self_contig=True)

    nc.compile()
    return nc


def _pad128(n):
    return max(_P, (n + _P - 1) // _P * _P)


def _plan(x, nbr0, nbr1, nbr2, weights):
    """Host-side sharding: per-core index tables + replicated weight uploads."""
    n_cores = _NCORES
    per = _OUT_PER_CORE
    cores = []
    for k in range(n_cores):
        out_ids = np.arange(k * per, (k + 1) * per, dtype=np.int64)
        l2n = nbr2[out_ids].astype(np.int64)  # [64, 5]
        # h1 table rows: out_ids first (layer-2 self rows become h1[0:64]),
        # then the remaining layer-1 dsts the subtree needs.
        need1 = np.concatenate(
            [out_ids, np.setdiff1d(l2n.ravel(), out_ids)]
        )
        inv1 = np.full(_N2, -1, np.int64)
        inv1[need1] = np.arange(len(need1))
        gidx2 = np.zeros((_P, _F2 + 1), np.int32)
        gidx2[:per, 0] = inv1[out_ids]
        gidx2[:per, 1:] = inv1[l2n]

        l1n = nbr1[need1].astype(np.int64)  # [u1, 10]
        # h0 table rows: need1 first in identical order (layer-1 self rows are
        # then the contiguous prefix of h0), then remaining layer-0 dsts.
        need0 = np.concatenate(
            [need1, np.setdiff1d(l1n.ravel(), need1)]
        )
        inv0 = np.full(_N1, -1, np.int64)
        inv0[need0] = np.arange(len(need0))
        u1 = len(need1)
        g1 = np.zeros((u1, _F1 + 1), np.int64)
        g1[:, 0] = inv0[need1]
        g1[:, 1:] = inv0[l1n]

        l0n = nbr0[need0].astype(np.int64)  # [u0, 10]
        u0 = len(need0)
        g0 = np.zeros((u0, _F0 + 1), np.int64)
        g0[:, 0] = need0
        g0[:, 1:] = l0n
        cores.append((gidx2, g1, g0, u1, u0))

    u1p = _pad128(max(c[3] for c in cores))
    u0p = _pad128(max(c[4] for c in cores))

    wcat0 = np.concatenate(
        [weights["Wself0"], weights["Wneigh0"] / _F0], axis=0
    ).astype(np.float32)
    wcat1 = np.concatenate(
        [weights["Wself1"], weights["Wneigh1"] / _F1], axis=0
    ).astype(np.float32)
    wcat2 = np.concatenate(
        [weights["Wself2"], weights["Wneigh2"] / _F2], axis=0
    ).astype(np.float32)
    b0 = weights["b0"].reshape(1, -1).astype(np.float32)
    b1 = weights["b1"].reshape(1, -1).astype(np.float32)
    b2 = weights["b2"].reshape(1, -1).astype(np.float32)

    in_maps = []
    for gidx2, g1, g0, u1, u0 in cores:
        G1 = np.zeros((u1p, _F1 + 1), np.int32)
        G1[:u1] = g1.astype(np.int32)
        G0 = np.zeros((u0p, _F0 + 1), np.int32)
        G0[:u0] = g0.astype(np.int32)
        in_maps.append(
            {
                "x": x,
                "gidx0": G0,
                "gidx1": G1,
                "gidx2": gidx2,
                "wcat0": wcat0,
                "wcat1": wcat1,
                "wcat2": wcat2,
                "bias0": b0,
                "bias1": b1,
                "bias2": b2,
            }
        )
    return in_maps, u0p, u1p


def _prepare(**inputs):
    x = np.ascontiguousarray(np.asarray(inputs["x"], dtype=np.float32))
    nbr0 = np.asarray(inputs["nbr0"])
    nbr1 = np.asarray(inputs["nbr1"])
    nbr2 = np.asarray(inputs["nbr2"])
    weights = {
        k: np.asarray(inputs[k], dtype=np.float32)
        for k in (
            "Wself0", "Wneigh0", "b0",
            "Wself1", "Wneigh1", "b1",
            "Wself2", "Wneigh2", "b2",
        )
    }
    in_maps, u0p, u1p = _plan(x, nbr0, nbr1, nbr2, weights)
    key = (u0p, u1p)
    if key not in _compiled:
        _compiled[key] = _build(u0p, u1p)
    return _compiled[key], in_maps


def kernel(**inputs) -> np.ndarray:
    from concourse.bass_utils import run_bass_kernel_spmd

    nc, in_maps = _prepare(**inputs)
    res = run_bass_kernel_spmd(nc, in_maps, core_ids=list(range(_NCORES)))
    out = np.concatenate(
        [res.results[k]["out"][:_OUT_PER_CORE] for k in range(_NCORES)], axis=0
    )
    return out.astype(np.float32)



# revision 10
# speedup vs baseline: 2.3951x; 2.3951x over previous
"""Trainium2 Bass kernel for nn_DistSAGE (3-layer GraphSAGE, mean aggregation).

Sharding: the ~13.8k layer-0 dst rows that the 512 output nodes transitively
need are deduplicated GLOBALLY and split contiguously across the 8 cores
(1792 rows each, vs ~3700 with per-core subtrees); an AllGather replicates
the finished h0 table (Shared-HBM output) and layers 1/2 then run per-core on
each core's own 64-output subtree with indices remapped into the global h0
ordering. All feature data moves as bf16 (host converts once); PSUM
accumulation stays f32, end-to-end error ~4e-3 vs the f32 reference.

Per 128-dst chunk: indirect DMAs gather the self + neighbor rows (2KB bf16
rows, one SWDGE instruction per slot - the HW ucode consumes exactly one
index per partition - spread over the 4 SWDGE queues), DVE tree-adds the
neighbors, the PE transposes [dst, feat] -> [feat, dst] through PSUM with the
scalar engine copying back out, and bf16 matmuls accumulate into PSUM (+bias
via a K=1 ones matmul), relu -> bf16 h scratch in DRAM.
"""

import numpy as np

_N0, _N1, _N2, _N3 = 256000, 25600, 2560, 512
_DIN, _DH, _DOUT = 1024, 256, 19
_F0, _F1, _F2 = 10, 10, 5
_NCORES = 8
_P = 128
_OUT_PER_CORE = _N3 // _NCORES  # 64

_compiled = {}


def _build(s0, u1p, repeat=1):
    import concourse.bass as bass
    import concourse.mybir as mybir
    import concourse.tile as tile
    from concourse import bacc
    from concourse.masks import make_identity

    P = _P
    C0, C1 = s0 // P, u1p // P
    nc = bacc.Bacc(
        "TRN2", target_bir_lowering=False, debug=False, num_devices=_NCORES,
        num_swdge_queues=4,
    )
    f32 = mybir.dt.float32
    bf16 = mybir.dt.bfloat16
    i32 = mybir.dt.int32

    x = nc.dram_tensor("x", [_N0, _DIN], bf16, kind="ExternalInput")
    idx0 = nc.dram_tensor("idx0", [P, C0 * (_F0 + 1)], i32, kind="ExternalInput")
    idx1 = nc.dram_tensor("idx1", [P, C1 * (_F1 + 1)], i32, kind="ExternalInput")
    idx2 = nc.dram_tensor("idx2", [P, _F2 + 1], i32, kind="ExternalInput")
    wcat0 = nc.dram_tensor("wcat0", [2 * _DIN, _DH], bf16, kind="ExternalInput")
    wcat1 = nc.dram_tensor("wcat1", [2 * _DH, _DH], bf16, kind="ExternalInput")
    wcat2 = nc.dram_tensor("wcat2", [2 * _DH, _DOUT], bf16, kind="ExternalInput")
    bias0 = nc.dram_tensor("bias0", [1, _DH], bf16, kind="ExternalInput")
    bias1 = nc.dram_tensor("bias1", [1, _DH], bf16, kind="ExternalInput")
    bias2 = nc.dram_tensor("bias2", [1, _DOUT], bf16, kind="ExternalInput")
    out = nc.dram_tensor("out", [P, _DOUT], f32, kind="ExternalOutput")

    h0loc = nc.dram_tensor("h0loc", [s0, _DH], bf16, kind="Internal")
    h0g = nc.dram_tensor(
        "h0g", [_NCORES * s0, _DH], bf16, kind="Internal", addr_space="Shared"
    )
    h1 = nc.dram_tensor("h1scratch", [u1p, _DH], bf16, kind="Internal")

    def qname(q):
        q = q % 4
        return "qPoolDynamic" if q == 0 else f"qPoolDynamic{q}"

    with tile.TileContext(nc) as tc:
        with (
            tc.tile_pool(name="const", bufs=1) as cpool,
            tc.tile_pool(name="gather", bufs=3) as gpool,
            tc.tile_pool(name="zt", bufs=2) as zpool,
            tc.tile_pool(name="outp", bufs=2) as opool,
            tc.tile_pool(name="psacc", bufs=2, space="PSUM") as psacc,
            tc.tile_pool(name="pstp", bufs=4, space="PSUM") as pstp,
        ):
            ident = cpool.tile([P, P], bf16)
            make_identity(nc, ident[:])
            ones = cpool.tile([1, P], bf16)
            nc.gpsimd.memset(ones[:], 1.0)

            # resident weights: k-chunk c of wcat lives at wt[:, c*DO:(c+1)*DO]
            def load_w(wdram, kd, do, name):
                wt = cpool.tile([P, kd // P * do], bf16, name=name)
                for k in range(kd // P):
                    nc.sync.dma_start(
                        out=wt[:, k * do : (k + 1) * do],
                        in_=wdram[k * P : (k + 1) * P, :],
                    )
                return wt

            wt0 = load_w(wcat0, 2 * _DIN, _DH, "wt0")
            wt1 = load_w(wcat1, 2 * _DH, _DH, "wt1")
            wt2 = load_w(wcat2, 2 * _DH, _DOUT, "wt2")
            bts = []
            for b_, do in ((bias0, _DH), (bias1, _DH), (bias2, _DOUT)):
                bt = cpool.tile([1, do], bf16)
                nc.sync.dma_start(out=bt[:], in_=b_[:])
                bts.append(bt)
            # packed per-chunk index tables, resident for the whole kernel
            it0 = cpool.tile([P, C0 * (_F0 + 1)], i32)
            nc.sync.dma_start(out=it0[:], in_=idx0[:])
            it1 = cpool.tile([P, C1 * (_F1 + 1)], i32)
            nc.sync.dma_start(out=it1[:], in_=idx1[:])
            it2 = cpool.tile([P, _F2 + 1], i32)
            nc.sync.dma_start(out=it2[:], in_=idx2[:])

            qctr = [0]

            def layer(src, it, nch, d, fan, wt, bt, do, relu, dst):
                w = fan + 1
                kc = 2 * d // P
                for c in range(nch):
                    g = gpool.tile([P, w * d], bf16, tag=f"g{d}")
                    for j in range(w):
                        ins = nc.gpsimd.indirect_dma_start(
                            out=g[:, j * d : (j + 1) * d],
                            out_offset=None,
                            in_=src[:],
                            in_offset=bass.IndirectOffsetOnAxis(
                                ap=it[:, c * w + j : c * w + j + 1], axis=0
                            ),
                        )
                        ins.ins.queue = qname(qctr[0])
                        qctr[0] += 1
                    # tree-sum the fan neighbor rows into g[:, d:2d]
                    if fan == 10:
                        nc.vector.tensor_add(out=g[:, d:6*d], in0=g[:, d:6*d], in1=g[:, 6*d:11*d])
                        nc.vector.tensor_add(out=g[:, d:3*d], in0=g[:, d:3*d], in1=g[:, 3*d:5*d])
                        nc.vector.tensor_add(out=g[:, d:2*d], in0=g[:, d:2*d], in1=g[:, 2*d:3*d])
                        nc.vector.tensor_add(out=g[:, d:2*d], in0=g[:, d:2*d], in1=g[:, 5*d:6*d])
                    elif fan == 5:
                        nc.vector.tensor_add(out=g[:, d:3*d], in0=g[:, d:3*d], in1=g[:, 3*d:5*d])
                        nc.vector.tensor_add(out=g[:, d:2*d], in0=g[:, d:2*d], in1=g[:, 2*d:3*d])
                        nc.vector.tensor_add(out=g[:, d:2*d], in0=g[:, d:2*d], in1=g[:, 5*d:6*d])
                    else:
                        raise NotImplementedError(fan)

                    # transpose [dst, feat] -> [feat, dst] on the PE; scalar
                    # engine copies each PSUM block back to SBUF
                    zt = zpool.tile([P, 2 * d], bf16, tag=f"zt{d}")
                    for k in range(kc):
                        tp = pstp.tile([P, P], bf16, tag="tp")
                        nc.tensor.transpose(
                            out=tp[:],
                            in_=g[:, k * P : (k + 1) * P],
                            identity=ident[:],
                        )
                        nc.scalar.activation(
                            out=zt[:, k * P : (k + 1) * P],
                            in_=tp[:],
                            func=mybir.ActivationFunctionType.Copy,
                        )

                    acc = psacc.tile([P, do], f32, tag="acc")
                    for k in range(kc):
                        nc.tensor.matmul(
                            out=acc[:],
                            lhsT=zt[:, k * P : (k + 1) * P],
                            rhs=wt[:, k * do : (k + 1) * do],
                            start=(k == 0),
                            stop=False,
                        )
                    nc.tensor.matmul(
                        out=acc[:], lhsT=ones[:], rhs=bt[:], start=False, stop=True
                    )
                    odt = f32 if dst is out else bf16
                    ot = opool.tile([P, do], odt, tag=f"ot{do}")
                    nc.scalar.activation(
                        out=ot[:],
                        in_=acc[:],
                        func=(
                            mybir.ActivationFunctionType.Relu
                            if relu
                            else mybir.ActivationFunctionType.Copy
                        ),
                    )
                    nc.sync.dma_start(out=dst[c * P : (c + 1) * P, :], in_=ot[:])

            def body():
                layer(x, it0, C0, _DIN, _F0, wt0, bts[0], _DH, True, h0loc)
                nc.gpsimd.collective_compute(
                    "AllGather",
                    mybir.AluOpType.bypass,
                    replica_groups=[list(range(_NCORES))],
                    ins=[h0loc[:].opt()],
                    outs=[h0g[:].opt()],
                )
                layer(h0g, it1, C1, _DH, _F1, wt1, bts[1], _DH, True, h1)
                layer(h1, it2, 1, _DH, _F2, wt2, bts[2], _DOUT, False, out)

            # timing-only variants unroll the body so (T(repeat) - T(1)) /
            # (repeat - 1) cancels the ~80ms axon dispatch floor and drift
            for _ in range(repeat):
                body()

    nc.compile()
    return nc


def _pad128(n):
    return max(_P, (n + _P - 1) // _P * _P)


def _pack_idx(g, n_pad, w):
    """[n_pad, w] row-major index table -> [128, (n_pad/128)*w] so chunk c's
    indices for dst partition p sit at [p, c*w:(c+1)*w]."""
    C = n_pad // _P
    return np.ascontiguousarray(
        g.reshape(C, _P, w).transpose(1, 0, 2).reshape(_P, C * w)
    )


def _plan(x, nbr0, nbr1, nbr2, weights):
    """Host-side sharding: global layer-0 dedup + per-core subtree tables."""
    import ml_dtypes

    bf16 = ml_dtypes.bfloat16
    n_cores = _NCORES
    per = _OUT_PER_CORE

    nbr0 = nbr0.astype(np.int64)
    nbr1 = nbr1.astype(np.int64)
    nbr2 = nbr2.astype(np.int64)

    # global layer-0 dedup: every h0 row any core will need, computed once
    need1_g = np.union1d(np.arange(_N3), nbr2.ravel())
    need0_g = np.union1d(need1_g, nbr1[need1_g].ravel())
    s0 = _pad128((len(need0_g) + n_cores - 1) // n_cores)
    n0g = n_cores * s0
    need0_pad = np.zeros(n0g, np.int64)
    need0_pad[: len(need0_g)] = need0_g
    inv0_g = np.full(_N1, -1, np.int64)
    inv0_g[need0_g] = np.arange(len(need0_g))

    cores = []
    for k in range(n_cores):
        sl = need0_pad[k * s0 : (k + 1) * s0]
        g0 = np.zeros((s0, _F0 + 1), np.int64)
        g0[:, 0] = sl
        g0[:, 1:] = nbr0[sl]

        out_ids = np.arange(k * per, (k + 1) * per, dtype=np.int64)
        l2n = nbr2[out_ids]
        need1 = np.concatenate([out_ids, np.setdiff1d(l2n.ravel(), out_ids)])
        inv1 = np.full(_N2, -1, np.int64)
        inv1[need1] = np.arange(len(need1))
        gidx2 = np.zeros((_P, _F2 + 1), np.int64)
        gidx2[:per, 0] = inv1[out_ids]
        gidx2[:per, 1:] = inv1[l2n]

        l1n = nbr1[need1]
        u1 = len(need1)
        g1 = np.zeros((u1, _F1 + 1), np.int64)
        g1[:, 0] = inv0_g[need1]
        g1[:, 1:] = inv0_g[l1n]
        cores.append((g0, g1, gidx2, u1))

    u1p = _pad128(max(c[3] for c in cores))

    wcat0 = np.concatenate([weights["Wself0"], weights["Wneigh0"] / _F0], axis=0).astype(bf16)
    wcat1 = np.concatenate([weights["Wself1"], weights["Wneigh1"] / _F1], axis=0).astype(bf16)
    wcat2 = np.concatenate([weights["Wself2"], weights["Wneigh2"] / _F2], axis=0).astype(bf16)
    b0 = weights["b0"].reshape(1, -1).astype(bf16)
    b1 = weights["b1"].reshape(1, -1).astype(bf16)
    b2 = weights["b2"].reshape(1, -1).astype(bf16)
    xb = np.ascontiguousarray(x.astype(bf16))

    in_maps = []
    for g0, g1, gidx2, u1 in cores:
        G1 = np.zeros((u1p, _F1 + 1), np.int64)
        G1[:u1] = g1
        in_maps.append(
            {
                "x": xb,
                "idx0": _pack_idx(g0.astype(np.int32), s0, _F0 + 1),
                "idx1": _pack_idx(G1.astype(np.int32), u1p, _F1 + 1),
                "idx2": _pack_idx(gidx2.astype(np.int32), _P, _F2 + 1),
                "wcat0": wcat0, "wcat1": wcat1, "wcat2": wcat2,
                "bias0": b0, "bias1": b1, "bias2": b2,
            }
        )
    return in_maps, s0, u1p


def _prepare(**inputs):
    x = np.ascontiguousarray(np.asarray(inputs["x"], dtype=np.float32))
    nbr0 = np.asarray(inputs["nbr0"])
    nbr1 = np.asarray(inputs["nbr1"])
    nbr2 = np.asarray(inputs["nbr2"])
    weights = {
        k: np.asarray(inputs[k], dtype=np.float32)
        for k in (
            "Wself0", "Wneigh0", "b0",
            "Wself1", "Wneigh1", "b1",
            "Wself2", "Wneigh2", "b2",
        )
    }
    in_maps, s0, u1p = _plan(x, nbr0, nbr1, nbr2, weights)
    key = (s0, u1p)
    if key not in _compiled:
        _compiled[key] = _build(s0, u1p)
    return _compiled[key], in_maps


def kernel(**inputs) -> np.ndarray:
    from concourse.bass_utils import run_bass_kernel_spmd

    nc, in_maps = _prepare(**inputs)
    res = run_bass_kernel_spmd(nc, in_maps, core_ids=list(range(_NCORES)))
    out = np.concatenate(
        [res.results[k]["out"][:_OUT_PER_CORE] for k in range(_NCORES)], axis=0
    )
    return out.astype(np.float32)
